# revision 1
# baseline (speedup 1.0000x reference)
"""Causal self-attention (B=2, T=2048, D=2048, H=16, HD=128) on 8 TRN2 cores.

Strategy: tensor-parallel over heads (2 heads/core) for QKV projection, RMS
norm, rotary, and attention; AllToAll reshards the attention output over
tokens; each core then runs the output projection for its 512-token slice.
All matmul contractions sit on the partition dim via host-side transposes:
  - qT/kT/vT come out of the QKV matmul as [outdim, token] tiles directly,
  - scores are computed transposed (S^T = krot^T.T @ qrot^T -> [k, q]), so
    the PV matmul needs no on-device transposes,
  - softmax denominator = all-ones matmul over expS^T (also acts as the
    partition-broadcast), normalization folds into the PSUM->SBUF copy.
Matmuls run in bf16 (fp32 is 4 cycles/row on the PE, bf16 is 1); PSUM
accumulation and softmax statistics stay fp32.
"""

import numpy as np

B, T, D = 2, 2048, 2048
H, HD = 16, 128
N_CORES = 8
HPC = H // N_CORES          # heads per core
NT = B * T                  # 4096 tokens, b-major
TS = NT // N_CORES          # 512-token output slice per core
DC = D // 128               # 16 contraction chunks
NTT = NT // 512             # 8 token tiles in phase 1
KT_PER_B = T // 128         # 16 k-tiles per batch row

_CACHE = {}


def _build(scale: float, reps: int = 1):
    import concourse.bacc as bacc
    import concourse.mybir as mybir
    import concourse.tile as tile

    f32 = mybir.dt.float32
    MM = mybir.dt.bfloat16
    EPS = float(np.finfo(np.float32).eps)

    nc = bacc.Bacc("TRN2", target_bir_lowering=False, debug=False,
                   num_devices=N_CORES)

    xT_d = nc.dram_tensor("xT", [D, NT], MM, kind="ExternalInput")
    wqk_d = nc.dram_tensor("wqk", [D, 4 * 128], MM, kind="ExternalInput")
    wv_d = nc.dram_tensor("wv", [D, HPC * HD], MM, kind="ExternalInput")
    wo_d = nc.dram_tensor("wo", [D, D], MM, kind="ExternalInput")
    cs_d = nc.dram_tensor("cs", [2, HD, NT], MM, kind="ExternalInput")
    mask_d = nc.dram_tensor("mask", [4, 128, 512], MM, kind="ExternalInput")
    ones_d = nc.dram_tensor("ones", [128, 128], MM, kind="ExternalInput")
    y_d = nc.dram_tensor("y", [TS, D], f32, kind="ExternalOutput")

    Sq = mybir.ActivationFunctionType.Square
    Sqrt = mybir.ActivationFunctionType.Sqrt
    Exp = mybir.ActivationFunctionType.Exp
    Copy = mybir.ActivationFunctionType.Copy
    mult = mybir.AluOpType.mult
    add = mybir.AluOpType.add

    with tile.TileContext(nc) as tc:
        with tc.tile_pool(name="dram", bufs=1, space="DRAM") as dram, \
             tc.tile_pool(name="res", bufs=1) as res:
            a2a_in_h = [dram.tile([N_CORES, HD, TS], MM, tag=f"a2a_in{h}",
                                  name=f"a2a_in{h}") for h in range(HPC)]
            a2a_out_h = [dram.tile([N_CORES, HD, TS], MM, tag=f"a2a_out{h}",
                                   name=f"a2a_out{h}") for h in range(HPC)]

            # Residents through phase 2: rotated q/k (m-chunks: q0,q1,k0,k1),
            # v in [token, hd] layout, causal masks, all-ones stationary.
            qk_sb = res.tile([128, 4 * NT], MM, tag="qk")
            v_sb = res.tile([128, (NT // 128) * (HPC * HD)], MM, tag="v")
            mask_sb = res.tile([128, 4 * 512], MM, tag="mask")
            ones_sb = res.tile([128, 128], MM, tag="ones")
            eps_sb = res.tile([128, 1], f32, tag="eps")
            nc.vector.memset(eps_sb[:], EPS)
            for m in range(4):
                nc.sync.dma_start(out=mask_sb[:, m * 512:(m + 1) * 512],
                                  in_=mask_d[m, :, :])
            nc.sync.dma_start(out=ones_sb[:], in_=ones_d[:, :])

            for _rep in range(reps):
                # ---------------- Phase 1: QKV + RMS norm + rotary ----------------
                with tc.tile_pool(name="p1", bufs=1) as p1, \
                     tc.tile_pool(name="xs", bufs=3) as xs, \
                     tc.tile_pool(name="st", bufs=3) as st, \
                     tc.tile_pool(name="ps1", bufs=2, space="PSUM") as ps1:
                    wqk_sb = p1.tile([128, DC * 512], MM, tag="wqk")
                    wv_sb = p1.tile([128, DC * HPC * HD], MM, tag="wv")
                    cs_sb = p1.tile([128, 2 * NT], MM, tag="cs")
                    nc.sync.dma_start(
                        out=wv_sb[:].rearrange("p (c f) -> p c f", f=256),
                        in_=wv_d[:, :].rearrange("(c p) f -> p c f", p=128))

                    for n in range(NTT):
                        xblk = xs.tile([128, DC * 512], MM, tag="xblk")
                        for cg in range(4):
                            nc.sync.dma_start(
                                out=xblk[:, cg * 4 * 512:(cg + 1) * 4 * 512]
                                    .rearrange("p (c f) -> p c f", f=512),
                                in_=xT_d[cg * 512:(cg + 1) * 512, n * 512:(n + 1) * 512]
                                    .rearrange("(c p) f -> p c f", p=128))
                        # v projection: [token, hd] layout
                        for c4 in range(4):
                            vps = ps1.tile([128, HPC * HD], f32, tag="vps")
                            for dc in range(DC):
                                nc.tensor.matmul(
                                    vps[:],
                                    xblk[:, dc * 512 + c4 * 128: dc * 512 + (c4 + 1) * 128],
                                    wv_sb[:, dc * 256:(dc + 1) * 256],
                                    start=(dc == 0), stop=(dc == DC - 1))
                            tcg = n * 4 + c4
                            nc.vector.tensor_copy(v_sb[:, tcg * 256:(tcg + 1) * 256], vps[:])
                        if n == 0:
                            # emitted late so the first v chain's loads go first
                            nc.sync.dma_start(
                                out=wqk_sb[:].rearrange("p (c f) -> p c f", f=512),
                                in_=wqk_d[:, :].rearrange("(c p) f -> p c f", p=128))
                            for s in range(2):
                                nc.sync.dma_start(out=cs_sb[:, s * NT:(s + 1) * NT],
                                                  in_=cs_d[s, :, :])
                        # q/k projection + rms + rotary, m-chunks q0,q1,k0,k1
                        for m in range(4):
                            qps = ps1.tile([128, 512], f32, tag="qps")
                            for dc in range(DC):
                                nc.tensor.matmul(
                                    qps[:],
                                    wqk_sb[:, dc * 512 + m * 128: dc * 512 + (m + 1) * 128],
                                    xblk[:, dc * 512:(dc + 1) * 512],
                                    start=(dc == 0), stop=(dc == DC - 1))
                            sq = st.tile([128, 512], MM, tag="sq")
                            nc.scalar.activation(sq[:], qps[:], Sq)
                            ssq = ps1.tile([128, 512], f32, tag="ssq")
                            nc.tensor.matmul(ssq[:], ones_sb[:], sq[:], start=True, stop=True)
                            rms = st.tile([128, 512], f32, tag="rms")
                            nc.scalar.activation(rms[:], ssq[:], Sqrt, bias=eps_sb[:], scale=1.0 / HD)
                            r = st.tile([128, 512], f32, tag="r")
                            nc.vector.reciprocal(r[:], rms[:])
                            qn = st.tile([128, 512], MM, tag="qn")
                            nc.vector.tensor_mul(qn[:], qps[:], r[:])
                            # rotary: y = qn*C + swap(qn)*S  with S = [sin; -sin]
                            tsw = st.tile([128, 512], MM, tag="tsw")
                            ctile = cs_sb[:, n * 512:(n + 1) * 512]
                            stile = cs_sb[:, NT + n * 512: NT + (n + 1) * 512]
                            # stile holds [-sin; sin]: each mul's inputs share a
                            # base partition; only the output is partition-shifted.
                            nc.vector.tensor_mul(tsw[0:64, :], qn[64:128, :], stile[64:128, :])
                            nc.vector.tensor_mul(tsw[64:128, :], qn[0:64, :], stile[0:64, :])
                            dst = qk_sb[:, m * NT + n * 512: m * NT + (n + 1) * 512]
                            nc.vector.tensor_mul(dst, qn[:], ctile)
                            nc.vector.tensor_add(dst, dst, tsw[:])

                # ---------------- Phase 2 + 3: attention, A2A, o-proj ----------------
                # h is the outer loop so head 0's AllToAll overlaps head 1's
                # attention; o-proj accumulates even d-chunks (head 0 data) first
                # so it can start before the second AllToAll lands.
                with tc.tile_pool(name="p2", bufs=4) as p2, \
                     tc.tile_pool(name="p2b", bufs=2) as p2b, \
                     tc.tile_pool(name="pss", bufs=2, space="PSUM") as pss, \
                     tc.tile_pool(name="psd", bufs=2, space="PSUM") as psd, \
                     tc.tile_pool(name="psy", bufs=2, space="PSUM") as psy, \
                     tc.tile_pool(name="p3", bufs=1) as p3, \
                     tc.tile_pool(name="wop", bufs=4) as wop, \
                     tc.tile_pool(name="ob", bufs=2) as obp, \
                     tc.tile_pool(name="prt", bufs=16) as prt, \
                     tc.tile_pool(name="ps3", bufs=2, space="PSUM") as ps3:
                    for h in range(HPC):
                        qoff = h * NT
                        koff = (2 + h) * NT
                        for b in range(B):
                            for qj in range(4):
                                yps = psy.tile([128, 512], f32, tag="yps")
                                dps = psd.tile([128, 512], f32, tag="dps")
                                nkt = 4 * qj + 4
                                qbase = qoff + b * T + qj * 512
                                for kb in range(nkt):
                                    # diagonal blocks: only q-columns >= 128*m live
                                    lo = max(0, (kb - 4 * qj) * 128)
                                    sps = pss.tile([128, 512], f32, tag="sps")
                                    nc.tensor.matmul(
                                        sps[:, lo:],
                                        qk_sb[:, koff + b * T + kb * 128: koff + b * T + (kb + 1) * 128],
                                        qk_sb[:, qbase + lo: qbase + 512],
                                        start=True, stop=True)
                                    e = p2.tile([128, 512], MM, tag="e")
                                    nc.scalar.activation(e[:, lo:], sps[:, lo:], Exp, scale=scale)
                                    if kb >= 4 * qj:
                                        mi = kb - 4 * qj
                                        nc.vector.tensor_mul(
                                            e[:, lo:], e[:, lo:],
                                            mask_sb[:, mi * 512 + lo:(mi + 1) * 512])
                                    nc.tensor.matmul(dps[:, lo:], ones_sb[:], e[:, lo:],
                                                     start=(kb == 0), stop=(kb == nkt - 1))
                                    tcg = b * KT_PER_B + kb
                                    nc.tensor.matmul(
                                        yps[:, lo:],
                                        v_sb[:, tcg * 256 + h * 128: tcg * 256 + (h + 1) * 128],
                                        e[:, lo:],
                                        start=(kb == 0), stop=(kb == nkt - 1))
                                rcp = p2b.tile([128, 512], f32, tag="rcp")
                                nc.vector.reciprocal(rcp[:], dps[:])
                                yn = p2b.tile([128, 512], MM, tag="yn")
                                nc.vector.tensor_mul(yn[:], yps[:], rcp[:])
                                s = b * 4 + qj
                                nc.sync.dma_start(out=a2a_in_h[h][s, :, :], in_=yn[:])
                        nc.gpsimd.collective_compute(
                            "AllToAll",
                            mybir.AluOpType.bypass,
                            replica_groups=[list(range(N_CORES))],
                            ins=[a2a_in_h[h].opt()],
                            outs=[a2a_out_h[h].opt()],
                        )

                    # o-proj: d-chunk dc2 = 2g + h lives in a2a_out_h[h][g];
                    # per-g DMAs so early chains need not wait for the full load
                    yT_h = []
                    for h in range(HPC):
                        yt = p3.tile([128, N_CORES * 512], MM, tag=f"yT{h}",
                                     name=f"yT{h}")
                        for g in range(N_CORES):
                            nc.sync.dma_start(out=yt[:, g * 512:(g + 1) * 512],
                                              in_=a2a_out_h[h][g, :, :])
                        yT_h.append(yt)
                    # all even (head-0) chains first, then all odd chains:
                    # keeps PSUM slot reuse from chaining evens behind odds
                    # that wait on the second collective
                    wo_blocks = []
                    for on in range(4):
                        wo_sb = wop.tile([128, DC * 512], MM, tag="wo")
                        for cg in range(4):
                            nc.sync.dma_start(
                                out=wo_sb[:, cg * 4 * 512:(cg + 1) * 4 * 512]
                                    .rearrange("p (c f) -> p c f", f=512),
                                in_=wo_d[cg * 512:(cg + 1) * 512, on * 512:(on + 1) * 512]
                                    .rearrange("(c p) f -> p c f", p=128))
                        wo_blocks.append(wo_sb)
                    parts = []
                    for on in range(4):
                        for mc in range(4):
                            pe_ps = ps3.tile([128, 512], f32, tag="ops")
                            for g in range(8):
                                nc.tensor.matmul(
                                    pe_ps[:],
                                    yT_h[0][:, g * 512 + mc * 128: g * 512 + (mc + 1) * 128],
                                    wo_blocks[on][:, 2 * g * 512:(2 * g + 1) * 512],
                                    start=(g == 0), stop=(g == 7))
                            part = prt.tile([128, 512], f32, tag="part")
                            nc.scalar.activation(part[:], pe_ps[:], Copy)
                            parts.append(part)
                    for on in range(4):
                        for mc in range(4):
                            po_ps = ps3.tile([128, 512], f32, tag="ops")
                            for g in range(8):
                                nc.tensor.matmul(
                                    po_ps[:],
                                    yT_h[1][:, g * 512 + mc * 128: g * 512 + (mc + 1) * 128],
                                    wo_blocks[on][:, (2 * g + 1) * 512:(2 * g + 2) * 512],
                                    start=(g == 0), stop=(g == 7))
                            ob = obp.tile([128, 512], f32, tag="ob")
                            nc.vector.tensor_add(ob[:], po_ps[:], parts[on * 4 + mc][:])
                            nc.sync.dma_start(
                                out=y_d[mc * 128:(mc + 1) * 128, on * 512:(on + 1) * 512],
                                in_=ob[:])

    nc.compile()
    return nc


def _prep_inputs(x, W, cos, sin):
    import concourse.mybir as mybir
    bf = mybir.dt.np(mybir.dt.bfloat16)

    xT = np.ascontiguousarray(x.reshape(NT, D).T).astype(bf)
    cT = cos.T.astype(np.float32)
    sT = sin.T.astype(np.float32)
    C128 = np.tile(np.concatenate([cT, cT], 0), (1, B)).astype(bf)
    S128 = np.tile(np.concatenate([-sT, sT], 0), (1, B)).astype(bf)
    cs = np.ascontiguousarray(np.stack([C128, S128]))
    masks = np.stack([
        (np.arange(128)[:, None] <= np.arange(512)[None, :] - 128 * m)
        for m in range(4)
    ]).astype(bf)
    ones = np.ones((128, 128), dtype=bf)
    wo = np.ascontiguousarray(W[3].T).astype(bf)

    in_maps = []
    for c in range(N_CORES):
        r0 = c * HPC * HD
        wqk = np.ascontiguousarray(
            np.concatenate([W[0][r0:r0 + 256], W[1][r0:r0 + 256]], 0).T).astype(bf)
        wv = np.ascontiguousarray(W[2][r0:r0 + 256].T).astype(bf)
        in_maps.append({
            "xT": xT, "wqk": wqk, "wv": wv, "wo": wo,
            "cs": cs, "mask": masks, "ones": ones,
        })
    return in_maps


def kernel(x, W, cos, sin, scale):
    from concourse.bass_utils import run_bass_kernel_spmd

    x = np.asarray(x, dtype=np.float32)
    W = np.asarray(W, dtype=np.float32)
    cos = np.asarray(cos, dtype=np.float32)
    sin = np.asarray(sin, dtype=np.float32)
    sc = float(np.asarray(scale))

    if sc not in _CACHE:
        _CACHE[sc] = _build(sc)
    nc = _CACHE[sc]

    in_maps = _prep_inputs(x, W, cos, sin)
    out = run_bass_kernel_spmd(nc, in_maps, core_ids=list(range(N_CORES)))
    y = np.concatenate([out.results[c]["y"] for c in range(N_CORES)], axis=0)
    return y.reshape(B, T, D)



# revision 8
# speedup vs baseline: 6.7566x; 6.7566x over previous
"""Causal self-attention (B=2, T=2048, D=2048, H=16, HD=128) on 8 TRN2 cores.

The per-execution cost on this stack is dominated by operand staging
(~0.4-0.6 ms per MB per core), so the kernel is designed to minimize staged
bytes: every tensor is sharded 8 ways host-side and the full activations are
reassembled on device with collectives.

  - x is sharded over tokens (2 MB/core, fp16) and AllGathered on device to
    the full [D, NT] layout each core needs for its heads.
  - W q/k/v rows are sharded over heads (3 MB/core), W_o is sharded over its
    input (head) dim (1 MB/core): each core computes an o-proj partial for
    ALL tokens and a ReduceScatter(add) sums partials while sharding tokens,
    writing the [512, D] fp16 output slice directly.
  - cos/sin are staged once as a [128, T] tile (0.5 MB), duplicated/negated
    on device; the causal mask triangle and the all-ones tile are built on
    device (memset/iota-free: 32 KB triangle staged as input).

All tensors and matmuls are fp16 (same PE throughput as bf16, 3 extra
mantissa bits); PSUM accumulation and softmax statistics stay fp32. Softmax
uses exp(s*scale - 6): the RMS norm bounds |s*scale| <= sqrt(128), so the
shift makes fp16 overflow impossible; the shift cancels in the p/sum(p)
normalization.

Total staged operands: ~6.5 MB/core inputs + 2 MB/core output buffer vs
~33.5 MB/core for the replicated baseline.
"""

import numpy as np

B, T, D = 2, 2048, 2048
H, HD = 16, 128
N_CORES = 8
HPC = H // N_CORES          # heads per core
NT = B * T                  # 4096 tokens, b-major
TS = NT // N_CORES          # 512-token shard per core
DC = D // 128               # 16 contraction chunks
NTT = NT // 512             # 8 token blocks
KT_PER_B = T // 128         # 16 k-tiles per batch row

_CACHE = {}


def _build(scale: float, reps: int = 1):
    import concourse.bacc as bacc
    import concourse.mybir as mybir
    import concourse.tile as tile

    f32 = mybir.dt.float32
    MM = mybir.dt.float16
    EPS = float(np.finfo(np.float32).eps)
    SHIFT = -6.0

    nc = bacc.Bacc("TRN2", target_bir_lowering=False, debug=False,
                   num_devices=N_CORES)

    # declaration order = staging order: x first so the AllGather can fire
    # while the weights are still staging
    xTs_d = nc.dram_tensor("xTs", [D, TS], MM, kind="ExternalInput")
    wqk_d = nc.dram_tensor("wqk", [D, 4 * 128], MM, kind="ExternalInput")
    wv_d = nc.dram_tensor("wv", [D, HPC * HD], MM, kind="ExternalInput")
    cs_d = nc.dram_tensor("cs", [128, T], MM, kind="ExternalInput")
    tri_d = nc.dram_tensor("tri", [128, 128], MM, kind="ExternalInput")
    wo_d = nc.dram_tensor("wo", [HPC * HD, D], MM, kind="ExternalInput")
    y_d = nc.dram_tensor("y", [TS, D], MM, kind="ExternalOutput")

    Sq = mybir.ActivationFunctionType.Square
    Sqrt = mybir.ActivationFunctionType.Sqrt
    Exp = mybir.ActivationFunctionType.Exp
    Copy = mybir.ActivationFunctionType.Copy

    with tile.TileContext(nc) as tc:
        with tc.tile_pool(name="dram", bufs=1, space="DRAM") as dram, \
             tc.tile_pool(name="res", bufs=1) as res:
            ag_in = dram.tile([D, TS], MM, tag="ag_in", name="ag_in")
            ag_out = dram.tile([NTT, D, TS], MM, tag="ag_out", name="ag_out",
                               addr_space="Shared")
            op_d = dram.tile([N_CORES, TS, D], MM, tag="op_d", name="op_d")
            rs_out = dram.tile([TS, D], MM, tag="rs_out", name="rs_out")

            # Residents: rotated q/k, v in [token, hd] layout, weights,
            # rotary tables, attention output (transposed), small constants.
            qk_sb = res.tile([128, 4 * NT], MM, tag="qk")
            v_sb = res.tile([128, (NT // 128) * (HPC * HD)], MM, tag="v")
            ynT_sb = res.tile([128, HPC * NT], MM, tag="ynT")
            wqk_sb = res.tile([128, DC * 512], MM, tag="wqk")
            wv_sb = res.tile([128, DC * HPC * HD], MM, tag="wv")
            wo_sb = res.tile([128, HPC * D], MM, tag="wo")
            c_sb = res.tile([128, T], MM, tag="c_sb")
            s_sb = res.tile([128, T], MM, tag="s_sb")
            tri_sb = res.tile([128, 128], MM, tag="tri")
            ones_sb = res.tile([128, 128], MM, tag="ones")
            eps_sb = res.tile([128, 1], f32, tag="eps")
            shift_sb = res.tile([128, 1], f32, tag="shift")
            nc.vector.memset(eps_sb[:], EPS)
            nc.vector.memset(shift_sb[:], SHIFT)
            nc.vector.memset(ones_sb[:], 1.0)

            for _rep in range(reps):
                # ---- AllGather x: [D, TS] per core -> [NTT, D, TS] ----
                # (collectives cannot read/write IO tensors; bounce via DRAM)
                nc.sync.dma_start(out=ag_in[:, :], in_=xTs_d[:, :])
                nc.gpsimd.collective_compute(
                    "AllGather",
                    mybir.AluOpType.bypass,
                    replica_groups=[list(range(N_CORES))],
                    ins=[ag_in.opt()],
                    outs=[ag_out.opt()],
                )

                # weight/table loads overlap the AllGather
                nc.sync.dma_start(
                    out=wqk_sb[:].rearrange("p (c f) -> p c f", f=512),
                    in_=wqk_d[:, :].rearrange("(c p) f -> p c f", p=128))
                nc.sync.dma_start(
                    out=wv_sb[:].rearrange("p (c f) -> p c f", f=256),
                    in_=wv_d[:, :].rearrange("(c p) f -> p c f", p=128))
                nc.sync.dma_start(
                    out=wo_sb[:].rearrange("p (c f) -> p c f", f=D),
                    in_=wo_d[:, :].rearrange("(c p) f -> p c f", p=128))
                nc.sync.dma_start(out=c_sb[0:64, :], in_=cs_d[0:64, :])
                nc.sync.dma_start(out=c_sb[64:128, :], in_=cs_d[0:64, :])
                nc.sync.dma_start(out=s_sb[64:128, :], in_=cs_d[64:128, :])
                ssrc = res.tile([64, T], MM, tag="ssrc")
                nc.sync.dma_start(out=ssrc[:], in_=cs_d[64:128, :])
                nc.scalar.activation(s_sb[0:64, :], ssrc[:], Copy, scale=-1.0)
                nc.sync.dma_start(out=tri_sb[:], in_=tri_d[:, :])

                # ---------------- Phase 1: QKV + RMS norm + rotary ----------------
                with tc.tile_pool(name="xs", bufs=3) as xs, \
                     tc.tile_pool(name="st", bufs=3) as st, \
                     tc.tile_pool(name="ps1", bufs=2, space="PSUM") as ps1:
                    for n in range(NTT):
                        xblk = xs.tile([128, DC * 512], MM, tag="xblk")
                        for cg in range(4):
                            nc.sync.dma_start(
                                out=xblk[:, cg * 4 * 512:(cg + 1) * 4 * 512]
                                    .rearrange("p (c f) -> p c f", f=512),
                                in_=ag_out[n, cg * 512:(cg + 1) * 512, :]
                                    .rearrange("(c p) f -> p c f", p=128))
                        # v projection: [token, hd] layout
                        for c4 in range(4):
                            vps = ps1.tile([128, HPC * HD], f32, tag="vps")
                            for dc in range(DC):
                                nc.tensor.matmul(
                                    vps[:],
                                    xblk[:, dc * 512 + c4 * 128: dc * 512 + (c4 + 1) * 128],
                                    wv_sb[:, dc * 256:(dc + 1) * 256],
                                    start=(dc == 0), stop=(dc == DC - 1))
                            tcg = n * 4 + c4
                            nc.vector.tensor_copy(v_sb[:, tcg * 256:(tcg + 1) * 256], vps[:])
                        # q/k projection + rms + rotary, m-chunks q0,q1,k0,k1
                        for m in range(4):
                            qps = ps1.tile([128, 512], f32, tag="qps")
                            for dc in range(DC):
                                nc.tensor.matmul(
                                    qps[:],
                                    wqk_sb[:, dc * 512 + m * 128: dc * 512 + (m + 1) * 128],
                                    xblk[:, dc * 512:(dc + 1) * 512],
                                    start=(dc == 0), stop=(dc == DC - 1))
                            sq = st.tile([128, 512], MM, tag="sq")
                            nc.scalar.activation(sq[:], qps[:], Sq)
                            ssq = ps1.tile([128, 512], f32, tag="ssq")
                            nc.tensor.matmul(ssq[:], ones_sb[:], sq[:], start=True, stop=True)
                            rms = st.tile([128, 512], f32, tag="rms")
                            nc.scalar.activation(rms[:], ssq[:], Sqrt, bias=eps_sb[:], scale=1.0 / HD)
                            r = st.tile([128, 512], f32, tag="r")
                            nc.vector.reciprocal(r[:], rms[:])
                            qn = st.tile([128, 512], MM, tag="qn")
                            nc.vector.tensor_mul(qn[:], qps[:], r[:])
                            # rotary: y = qn*C + swap(qn)*S  with S = [-sin; sin]
                            tsw = st.tile([128, 512], MM, tag="tsw")
                            tt = (n % 4) * 512
                            ctile = c_sb[:, tt:tt + 512]
                            stile = s_sb[:, tt:tt + 512]
                            nc.vector.tensor_mul(tsw[0:64, :], qn[64:128, :], stile[64:128, :])
                            nc.vector.tensor_mul(tsw[64:128, :], qn[0:64, :], stile[0:64, :])
                            dst = qk_sb[:, m * NT + n * 512: m * NT + (n + 1) * 512]
                            nc.vector.tensor_mul(dst, qn[:], ctile)
                            nc.vector.tensor_add(dst, dst, tsw[:])

                # ---------------- Phase 2: attention ----------------
                with tc.tile_pool(name="p2", bufs=4) as p2, \
                     tc.tile_pool(name="p2b", bufs=2) as p2b, \
                     tc.tile_pool(name="pss", bufs=2, space="PSUM") as pss, \
                     tc.tile_pool(name="psd", bufs=2, space="PSUM") as psd, \
                     tc.tile_pool(name="psy", bufs=2, space="PSUM") as psy:
                    for h in range(HPC):
                        qoff = h * NT
                        koff = (2 + h) * NT
                        for b in range(B):
                            for qj in range(4):
                                yps = psy.tile([128, 512], f32, tag="yps")
                                dps = psd.tile([128, 512], f32, tag="dps")
                                nkt = 4 * qj + 4
                                qbase = qoff + b * T + qj * 512
                                for kb in range(nkt):
                                    # diagonal blocks: only q-columns >= 128*mi live
                                    lo = max(0, (kb - 4 * qj) * 128)
                                    sps = pss.tile([128, 512], f32, tag="sps")
                                    nc.tensor.matmul(
                                        sps[:, lo:],
                                        qk_sb[:, koff + b * T + kb * 128: koff + b * T + (kb + 1) * 128],
                                        qk_sb[:, qbase + lo: qbase + 512],
                                        start=True, stop=True)
                                    e = p2.tile([128, 512], MM, tag="e")
                                    nc.scalar.activation(e[:, lo:], sps[:, lo:], Exp,
                                                         bias=shift_sb[:], scale=scale)
                                    if kb >= 4 * qj:
                                        # triangle mask on the diagonal 128 cols
                                        nc.vector.tensor_mul(
                                            e[:, lo:lo + 128], e[:, lo:lo + 128], tri_sb[:])
                                    nc.tensor.matmul(dps[:, lo:], ones_sb[:], e[:, lo:],
                                                     start=(kb == 0), stop=(kb == nkt - 1))
                                    tcg = b * KT_PER_B + kb
                                    nc.tensor.matmul(
                                        yps[:, lo:],
                                        v_sb[:, tcg * 256 + h * 128: tcg * 256 + (h + 1) * 128],
                                        e[:, lo:],
                                        start=(kb == 0), stop=(kb == nkt - 1))
                                rcp = p2b.tile([128, 512], f32, tag="rcp")
                                nc.vector.reciprocal(rcp[:], dps[:])
                                dst = ynT_sb[:, h * NT + b * T + qj * 512:
                                             h * NT + b * T + (qj + 1) * 512]
                                nc.vector.tensor_mul(dst, yps[:], rcp[:])

                # ---------------- Phase 3: o-proj partials + ReduceScatter ----------------
                with tc.tile_pool(name="p3", bufs=4) as p3, \
                     tc.tile_pool(name="ps3", bufs=4, space="PSUM") as ps3:
                    for tcg in range(NT // 128):
                        g, rr = tcg // 4, tcg % 4
                        for on in range(4):
                            ops = ps3.tile([128, 512], f32, tag="ops")
                            for h in range(HPC):
                                nc.tensor.matmul(
                                    ops[:],
                                    ynT_sb[:, h * NT + tcg * 128:
                                           h * NT + (tcg + 1) * 128],
                                    wo_sb[:, h * D + on * 512:h * D + (on + 1) * 512],
                                    start=(h == 0), stop=(h == HPC - 1))
                            pp = p3.tile([128, 512], MM, tag="pp")
                            nc.scalar.activation(pp[:], ops[:], Copy)
                            nc.sync.dma_start(
                                out=op_d[g, rr * 128:(rr + 1) * 128,
                                         on * 512:(on + 1) * 512],
                                in_=pp[:])
                    nc.gpsimd.collective_compute(
                        "ReduceScatter",
                        mybir.AluOpType.add,
                        replica_groups=[list(range(N_CORES))],
                        ins=[op_d.opt()],
                        outs=[rs_out.opt()],
                    )
                    nc.sync.dma_start(out=y_d[:, :], in_=rs_out.opt())

    nc.compile()
    return nc


def _prep_inputs(x, W, cos, sin):
    import concourse.mybir as mybir
    fp = mybir.dt.np(mybir.dt.float16)

    xT = np.ascontiguousarray(x.reshape(NT, D).T).astype(fp)
    cs = np.concatenate([cos.T, sin.T], 0).astype(fp)   # [128, T]
    tri = (np.arange(128)[:, None] <= np.arange(128)[None, :]).astype(fp)
    in_maps = []
    for c in range(N_CORES):
        r0 = c * HPC * HD
        wqk = np.ascontiguousarray(
            np.concatenate([W[0][r0:r0 + 256], W[1][r0:r0 + 256]], 0).T).astype(fp)
        wv = np.ascontiguousarray(W[2][r0:r0 + 256].T).astype(fp)
        wo = np.ascontiguousarray(W[3][:, r0:r0 + 256].T).astype(fp)
        in_maps.append({
            "xTs": np.ascontiguousarray(xT[:, c * TS:(c + 1) * TS]),
            "wqk": wqk, "wv": wv, "wo": wo, "cs": cs, "tri": tri,
        })
    return in_maps


def kernel(x, W, cos, sin, scale):
    from concourse.bass_utils import run_bass_kernel_spmd

    x = np.asarray(x, dtype=np.float32)
    W = np.asarray(W, dtype=np.float32)
    cos = np.asarray(cos, dtype=np.float32)
    sin = np.asarray(sin, dtype=np.float32)
    sc = float(np.asarray(scale))

    if sc not in _CACHE:
        _CACHE[sc] = _build(sc)
    nc = _CACHE[sc]

    in_maps = _prep_inputs(x, W, cos, sin)
    out = run_bass_kernel_spmd(nc, in_maps, core_ids=list(range(N_CORES)))
    y = np.concatenate([out.results[c]["y"] for c in range(N_CORES)], axis=0)
    return y.astype(np.float32).reshape(B, T, D)
